# revision 15
# baseline (speedup 1.0000x reference)
"""Trainium2 Bass kernel for CBCForecasterV2.

Model: 2-layer LSTM encoder (T=3) + baseline MLP/LayerNorm + 5-step
attention decoder (Bahdanau) with LSTMCell + output MLP.

Strategy
--------
Pure data parallelism: batch 32768 split across 8 cores (4096 each),
weights replicated.  On-core layout is feature-major: features on SBUF
partitions, batch samples along the free dimension, processed in tiles
of 512 samples (8 tiles per core).  All matmuls run as float32r
(full-rate at N=512).  All transcendentals come from the single
`exp_and_others` ACT table set (tanh/exp/relu/identity/square/copy) so
the scalar engine never switches spline tables:

 * sigmoid(x) is computed as tanh(0.5 x) with a doubled cell state:
     C := 2c ;  C' = (t_f+1)*c + (t_i+1)*t_g  via two fused
   scalar_tensor_tensor ops;  h is kept doubled (H := 2h) and every
   weight matrix that consumes h is pre-halved on the host.
 * attention scores are computed with `va` replicated across all 128
   output columns so the scores come out of the PE already broadcast
   over partitions; softmax normalisation is folded into the context
   sum as  ctx = enc2 + w0*(enc0-enc2) + w1*(enc1-enc2).
 * LayerNorm rsqrt uses the int32 magic-constant seed + 2 Newton
   iterations on the vector engine (no sqrt table needed).
"""

import numpy as np

import concourse.bass as bass
import concourse.bacc as bacc
import concourse.tile as tile
from concourse import mybir
from concourse.bass_utils import run_bass_kernel_spmd

F32 = mybir.dt.float32
F32R = mybir.dt.float32r
I32 = mybir.dt.int32
AF = mybir.ActivationFunctionType
ALU = mybir.AluOpType

B, T_IN, F, NB = 32768, 3, 6, 10
H = 256
BD = 128
T_OUT = 5
N_CORES = 8
B_CORE = B // N_CORES  # 4096
COLS = 512             # samples per batch tile (free dim of every op)
EPS = 1e-5

LAST_EXEC_NS = None
LAST_RESULTS = None


def _r(ap):
    return ap.bitcast(F32R)


def _gate_bias(b):
    # z gate order i,f,g,o; i/f/o go through tanh(0.5x+0.5b), g through tanh
    s = np.asarray(b, np.float64).copy()
    s[0:512] *= 0.5
    s[768:1024] *= 0.5
    return s.astype(np.float32)


def prep_weights(inputs):
    """Host-side weight prep: transpose to [K, M] (lhsT) layout, halve the
    matrices that consume doubled-h, fold biases."""
    f32 = np.float32
    g = lambda k: np.asarray(inputs[k], np.float64)
    w = {}
    w["wih0"] = g("W_ih0").T.astype(f32)                    # [6,1024]
    w["whh0"] = (g("W_hh0").T * 0.5).astype(f32)            # [256,1024]
    w["wih1"] = (g("W_ih1").T * 0.5).astype(f32)            # [256,1024]
    w["whh1"] = (g("W_hh1").T * 0.5).astype(f32)            # [256,1024]
    w["b0"] = _gate_bias(g("b_ih0") + g("b_hh0"))           # [1024]
    w["b1"] = _gate_bias(g("b_ih1") + g("b_hh1"))           # [1024]
    w["wb"] = g("Wb").T.astype(f32)                         # [10,128]
    w["bb"] = g("bb").astype(f32)                           # [128]
    w["lng"] = g("ln_g").astype(f32)                        # [128]
    w["lnb"] = g("ln_b").astype(f32)                        # [128]
    w["w1a"] = (g("W1a").T * 0.5).astype(f32)               # [256,256]
    w["w2a"] = (g("W2a").T * 0.5).astype(f32)               # [256,256]
    w["bqa"] = (g("b1a") + g("b2a")).astype(f32)            # [256]
    w["va"] = np.repeat(g("va").reshape(H, 1), 128, 1).astype(f32)  # [256,128]
    wd = g("Wd_ih")                                         # [1024,390]
    w["wdcbc"] = wd[:, 0:F].T.astype(f32)                   # [6,1024]
    w["wdctx"] = (wd[:, F:F + H].T * 0.5).astype(f32)       # [256,1024]
    w["wdbase"] = wd[:, F + H:].T.astype(f32)               # [128,1024]
    w["wdhh"] = (g("Wd_hh").T * 0.5).astype(f32)            # [256,1024]
    w["bd"] = _gate_bias(g("bd_ih") + g("bd_hh"))           # [1024]
    w["wo1"] = (g("Wo1").T * 0.5).astype(f32)               # [256,128]
    w["bo1"] = g("bo1").astype(f32)                         # [128]
    w["wo2"] = g("Wo2").T.astype(f32)                       # [128,6]
    w["bo2"] = g("bo2").astype(f32)                         # [6]
    w["wones"] = np.full((128, 128), 1.0 / BD, np.float32)
    return w


WEIGHT_SHAPES = {
    "wih0": [F, 4 * H], "whh0": [H, 4 * H], "wih1": [H, 4 * H],
    "whh1": [H, 4 * H], "b0": [4 * H], "b1": [4 * H],
    "wb": [NB, BD], "bb": [BD], "lng": [BD], "lnb": [BD],
    "w1a": [H, H], "w2a": [H, H], "bqa": [H], "va": [H, 128],
    "wdcbc": [F, 4 * H], "wdctx": [H, 4 * H], "wdbase": [BD, 4 * H],
    "wdhh": [H, 4 * H], "bd": [4 * H],
    "wo1": [H, BD], "bo1": [BD], "wo2": [BD, F], "bo2": [F],
    "wones": [128, 128],
}


def build_nc(n_bt, cols):
    """Build the Bass module processing n_bt batch tiles of `cols` samples."""
    nc = bacc.Bacc()
    bc = n_bt * cols

    dx = nc.declare_dram_parameter("x", [T_IN * F, bc], F32R, isOutput=False)
    dbas = nc.declare_dram_parameter("bas", [NB, bc], F32R, isOutput=False)
    _BIAS_NAMES = {"b0", "b1", "bd", "bqa", "bb", "lng", "lnb", "bo1", "bo2"}
    dw = {k: nc.declare_dram_parameter(
              k, list(s), F32 if k in _BIAS_NAMES else F32R, isOutput=False)
          for k, s in WEIGHT_SHAPES.items()}
    dout = nc.declare_dram_parameter("out", [T_OUT * F, bc], F32R, isOutput=True)

    from contextlib import ExitStack
    with tile.TileContext(nc) as tc, ExitStack() as _es:
        if True:
            _p = lambda **kw: _es.enter_context(tc.tile_pool(**kw))
            wp = _p(name="wp", bufs=1)
            iox = _p(name="iox", bufs=2)
            encp = _p(name="encp", bufs=6)
            prjp = _p(name="prjp", bufs=6)
            dpl = _p(name="dp", bufs=4)
            stH = _p(name="stH", bufs=3)
            stC = _p(name="stC", bufs=2)
            gp = _p(name="gp", bufs=8)
            ap4 = _p(name="ap4", bufs=4)
            ap3 = _p(name="ap3", bufs=3)
            ap2 = _p(name="ap2", bufs=2)
            bp = _p(name="bp", bufs=1)
            brp = _p(name="brp", bufs=2)
            prd = _p(name="prd", bufs=2)
            pz = _p(name="pz", bufs=4, space="PSUM")
            pq = _p(name="pq", bufs=2, space="PSUM")
            ps = _p(name="ps", bufs=2, space="PSUM")
            # ---------------- weights into SBUF (once) ----------------
            def load(name):
                d = dw[name]
                if d.shape[0] <= 128:
                    t = wp.tile(list(d.shape), F32R, tag=name)
                    nc.sync.dma_start(out=t, in_=d[:])
                    return t
                chunks = []
                for k in range(d.shape[0] // 128):
                    t = wp.tile([128, d.shape[1]], F32R, tag=f"{name}{k}")
                    nc.sync.dma_start(out=t, in_=d[128 * k:128 * (k + 1), :])
                    chunks.append(t)
                return chunks

            # K=6 operands must sit at base partition 0/32/64 and lhsT must
            # share the rhs base partition -> replicate the tiny K=6 weights
            # at those offsets.
            wih0 = wp.tile([70, 4 * H], F32R, tag="wih0")
            for t in range(T_IN):
                nc.sync.dma_start(out=wih0[32 * t:32 * t + F, :],
                                  in_=dw["wih0"][:])
            wdcbc = wp.tile([F, 4 * H], F32R, tag="wdcbc")
            nc.sync.dma_start(out=wdcbc, in_=dw["wdcbc"][:])
            whh0 = load("whh0")
            wih1 = load("wih1")
            whh1 = load("whh1")
            wbt = load("wb")
            w1a = load("w1a")
            w2a = load("w2a")
            vat = load("va")
            wdctx = load("wdctx")
            wdbase = load("wdbase")
            wdhh = load("wdhh")
            wo1 = load("wo1")
            wo2 = load("wo2")

            def loadb(name, p, m):
                t = wp.tile([p, m], F32, tag=f"b_{name}")
                nc.sync.dma_start(
                    out=t, in_=dw[name][:].rearrange("(m p) -> p m", p=p))
                return t

            b0t = loadb("b0", 128, 8)
            b1t = loadb("b1", 128, 8)
            bdt = loadb("bd", 128, 8)
            bqat = loadb("bqa", 128, 2)
            bbt = loadb("bb", 128, 1)
            lngt = loadb("lng", 128, 1)
            lnbt = loadb("lnb", 128, 1)
            bo1t = loadb("bo1", 128, 1)
            bo2t = loadb("bo2", F, 1)

            ones_t = wp.tile([128, 128], F32R, tag="ones")
            nc.sync.dma_start(out=ones_t, in_=dw["wones"][:])
            epst = wp.tile([128, 1], F32, tag="eps")
            nc.vector.memset(epst, EPS)

            # ---------------- per batch-tile program ----------------
            def lstm_cell(zts, cprev, bt, first, hpool, htag, cpool, ctag):
                """zts: 8 PSUM tiles of z (pre-bias). Returns (H2, C2) half
                tile pairs; H is doubled-h, C doubled-c."""
                hn, cn = [], []
                for h in (0, 1):
                    zi, zf, zg, zo = zts[0 + h], zts[2 + h], zts[4 + h], zts[6 + h]
                    ti = gp.tile([128, cols], F32, tag="gt")
                    nc.scalar.activation(ti, zi, AF.Tanh,
                                         bias=bt[:, 0 + h:1 + h], scale=0.5)
                    tg_ = gp.tile([128, cols], F32, tag="gt")
                    nc.scalar.activation(tg_, zg, AF.Tanh,
                                         bias=bt[:, 4 + h:5 + h], scale=1.0)
                    cnew = cpool.tile([128, cols], F32, tag=ctag)
                    if first:
                        # c0 = 0:  C' = (t_i+1)*t_g
                        nc.vector.scalar_tensor_tensor(
                            cnew, ti, 1.0, tg_, ALU.add, ALU.mult)
                    else:
                        tf = gp.tile([128, cols], F32, tag="gt")
                        nc.scalar.activation(tf, zf, AF.Tanh,
                                             bias=bt[:, 2 + h:3 + h], scale=0.5)
                        u = gp.tile([128, cols], F32, tag="gt")
                        nc.vector.scalar_tensor_tensor(
                            u, ti, 1.0, tg_, ALU.add, ALU.mult)
                        v = gp.tile([128, cols], F32, tag="gt")
                        nc.vector.scalar_tensor_tensor(
                            v, tf, 1.0, cprev[h], ALU.add, ALU.mult)
                        nc.vector.scalar_tensor_tensor(
                            cnew, v, 0.5, u, ALU.mult, ALU.add)
                    to = gp.tile([128, cols], F32, tag="gt")
                    nc.scalar.activation(to, zo, AF.Tanh,
                                         bias=bt[:, 6 + h:7 + h], scale=0.5)
                    th = gp.tile([128, cols], F32, tag="gt")
                    nc.scalar.activation(th, cnew, AF.Tanh, scale=0.5)
                    hnew = hpool.tile([128, cols], F32R, tag=htag)
                    nc.vector.scalar_tensor_tensor(
                        hnew, to, 1.0, th, ALU.add, ALU.mult)
                    hn.append(hnew)
                    cn.append(cnew)
                return hn, cn

            for j in range(n_bt):
                cs = slice(j * cols, (j + 1) * cols)
                xa = iox.tile([70, cols], F32R, tag="xa")
                for t in range(T_IN):
                    nc.sync.dma_start(out=xa[32 * t:32 * t + F, :],
                                      in_=dx[F * t:F * (t + 1), cs])
                bas = iox.tile([NB, cols], F32R, tag="bas")
                nc.sync.dma_start(out=bas, in_=dbas[:, cs])

                # ---------------- encoder ----------------
                h0 = c0 = h1 = c1 = None
                ench = []
                for t in range(T_IN):
                    xt = xa[32 * t:32 * t + F, :]
                    zts = []
                    for m in range(8):
                        ms = slice(128 * m, 128 * (m + 1))
                        zp = pz.tile([128, cols], F32, tag="z")
                        nc.tensor.matmul(zp, (wih0[32 * t:32 * t + F, ms]),
                                         (xt), start=True, stop=(t == 0))
                        if t > 0:
                            nc.tensor.matmul(zp, (whh0[0][:, ms]), (h0[0]),
                                             start=False, stop=False)
                            nc.tensor.matmul(zp, (whh0[1][:, ms]), (h0[1]),
                                             start=False, stop=True)
                        zts.append(zp)
                    h0, c0 = lstm_cell(zts, c0, b0t, t == 0, stH, "H0", stC, "C0")

                    zts = []
                    for m in range(8):
                        ms = slice(128 * m, 128 * (m + 1))
                        zp = pz.tile([128, cols], F32, tag="z")
                        nc.tensor.matmul(zp, (wih1[0][:, ms]), (h0[0]),
                                         start=True, stop=False)
                        nc.tensor.matmul(zp, (wih1[1][:, ms]), (h0[1]),
                                         start=False, stop=(t == 0))
                        if t > 0:
                            nc.tensor.matmul(zp, (whh1[0][:, ms]), (h1[0]),
                                             start=False, stop=False)
                            nc.tensor.matmul(zp, (whh1[1][:, ms]), (h1[1]),
                                             start=False, stop=True)
                        zts.append(zp)
                    h1, c1 = lstm_cell(zts, c1, b1t, t == 0, encp, "enc", stC, "C1")
                    ench.append(h1)

                # enc_proj (bias folded into attention tanh)
                proj = []
                for t in range(T_IN):
                    pr = []
                    for mh in (0, 1):
                        mslc = slice(128 * mh, 128 * (mh + 1))
                        pp = pq.tile([128, cols], F32, tag="pq")
                        nc.tensor.matmul(pp, (w2a[0][:, mslc]), (ench[t][0]),
                                         start=True, stop=False)
                        nc.tensor.matmul(pp, (w2a[1][:, mslc]), (ench[t][1]),
                                         start=False, stop=True)
                        pj = prjp.tile([128, cols], F32, tag="proj")
                        nc.vector.tensor_copy(out=pj, in_=pp)
                        pr.append(pj)
                    proj.append(pr)

                # d0 = enc0 - enc2, d1 = enc1 - enc2 (softmax folding)
                d0, d1 = [], []
                for hh in (0, 1):
                    a = dpl.tile([128, cols], F32, tag="dd")
                    nc.vector.tensor_sub(a, ench[0][hh], ench[2][hh])
                    d0.append(a)
                    b = dpl.tile([128, cols], F32, tag="dd")
                    nc.vector.tensor_sub(b, ench[1][hh], ench[2][hh])
                    d1.append(b)

                # ---------------- baseline encoder ----------------
                zb = pq.tile([128, cols], F32, tag="pq")
                nc.tensor.matmul(zb, (wbt), (bas), start=True, stop=True)
                rb = bp.tile([128, cols], F32R, tag="rb")
                nc.scalar.activation(rb, zb, AF.Relu, bias=bbt[:, 0:1])
                mu = pq.tile([128, cols], F32, tag="pq")
                nc.tensor.matmul(mu, (ones_t), (rb), start=True, stop=True)
                rc = bp.tile([128, cols], F32, tag="rc")
                nc.vector.tensor_sub(rc, rb, mu)
                sq = bp.tile([128, cols], F32R, tag="sq")
                nc.scalar.activation(sq, rc, AF.Square)
                vv = pq.tile([128, cols], F32, tag="pq")
                nc.tensor.matmul(vv, (ones_t), (sq), start=True, stop=True)
                vs = bp.tile([128, cols], F32, tag="vs")
                nc.scalar.activation(vs, vv, AF.Identity, bias=epst[:, 0:1])
                # rsqrt(vs): magic-constant seed + 2 Newton iterations
                y = bp.tile([128, cols], F32, tag="y0")
                nc.vector.tensor_scalar(
                    out=y.bitcast(I32), in0=vs.bitcast(I32),
                    scalar1=1, scalar2=None, op0=ALU.arith_shift_right)
                nc.vector.tensor_scalar(
                    out=y.bitcast(I32), in0=y.bitcast(I32),
                    scalar1=-1, scalar2=0x5F3759DF, op0=ALU.mult, op1=ALU.add)
                for it in range(2):
                    a = bp.tile([128, cols], F32, tag="nra")
                    nc.vector.tensor_mul(a, y, y)
                    bq = bp.tile([128, cols], F32, tag="nrb")
                    nc.vector.scalar_tensor_tensor(
                        bq, vs, -0.5, a, ALU.mult, ALU.mult)
                    nc.vector.scalar_tensor_tensor(
                        y, bq, 1.5, y, ALU.add, ALU.mult)
                brep = brp.tile([128, cols], F32R, tag="brep")
                nc.vector.tensor_mul(brep, rc, y)
                nc.vector.tensor_scalar(
                    out=brep, in0=brep, scalar1=lngt[:, 0:1],
                    scalar2=lnbt[:, 0:1], op0=ALU.mult, op1=ALU.add)

                # ---------------- decoder ----------------
                hd, cd = ench[2], c1
                lc = prd.tile([F, cols], F32R, tag="pred")
                nc.gpsimd.tensor_copy(out=lc, in_=xa[64:64 + F, :])
                for s in range(T_OUT):
                    # q = hd @ W1a.T (halved weights, doubled h)
                    qps = []
                    for mh in (0, 1):
                        mslc = slice(128 * mh, 128 * (mh + 1))
                        qp = pq.tile([128, cols], F32, tag="pq")
                        nc.tensor.matmul(qp, (w1a[0][:, mslc]), (hd[0]),
                                         start=True, stop=False)
                        nc.tensor.matmul(qp, (w1a[1][:, mslc]), (hd[1]),
                                         start=False, stop=True)
                        qps.append(qp)
                    # tanh(q + proj_t + bqa), then scores via replicated va
                    es = []
                    for t in range(T_IN):
                        tts = []
                        for mh in (0, 1):
                            tin = ap4.tile([128, cols], F32, tag="at")
                            nc.vector.tensor_add(tin, qps[mh], proj[t][mh])
                            tt_ = ap4.tile([128, cols], F32R, tag="at")
                            nc.scalar.activation(tt_, tin, AF.Tanh,
                                                 bias=bqat[:, mh:mh + 1])
                            tts.append(tt_)
                        sp = ps.tile([128, cols], F32, tag="ps")
                        nc.tensor.matmul(sp, (vat[0]), (tts[0]),
                                         start=True, stop=False)
                        nc.tensor.matmul(sp, (vat[1]), (tts[1]),
                                         start=False, stop=True)
                        e = ap3.tile([128, cols], F32, tag="e")
                        nc.scalar.activation(e, sp, AF.Exp)
                        es.append(e)
                    dsum = ap2.tile([128, cols], F32, tag="dr")
                    nc.vector.tensor_add(dsum, es[0], es[1])
                    nc.vector.tensor_add(dsum, dsum, es[2])
                    rr = ap2.tile([128, cols], F32, tag="dr")
                    nc.vector.reciprocal(out=rr, in_=dsum)
                    w0 = ap2.tile([128, cols], F32, tag="w")
                    nc.vector.tensor_mul(w0, es[0], rr)
                    w1_ = ap2.tile([128, cols], F32, tag="w")
                    nc.vector.tensor_mul(w1_, es[1], rr)
                    ctx = []
                    for hh in (0, 1):
                        t1_ = ap2.tile([128, cols], F32, tag="ct")
                        nc.vector.tensor_mul(t1_, w0, d0[hh])
                        t2_ = ap2.tile([128, cols], F32, tag="ct")
                        nc.vector.tensor_add(t2_, ench[2][hh], t1_)
                        t3_ = ap2.tile([128, cols], F32, tag="ct")
                        nc.vector.tensor_mul(t3_, w1_, d1[hh])
                        cx = ap2.tile([128, cols], F32R, tag="ctx")
                        nc.vector.tensor_add(cx, t2_, t3_)
                        ctx.append(cx)
                    # decoder LSTM cell
                    zts = []
                    for m in range(8):
                        ms = slice(128 * m, 128 * (m + 1))
                        zp = pz.tile([128, cols], F32, tag="z")
                        nc.tensor.matmul(
                            zp, (wdcbc[0:F, ms]), (lc),
                            start=True, stop=False)
                        nc.tensor.matmul(zp, (wdctx[0][:, ms]), (ctx[0]),
                                         start=False, stop=False)
                        nc.tensor.matmul(zp, (wdctx[1][:, ms]), (ctx[1]),
                                         start=False, stop=False)
                        nc.tensor.matmul(zp, (wdbase[:, ms]), (brep),
                                         start=False, stop=False)
                        nc.tensor.matmul(zp, (wdhh[0][:, ms]), (hd[0]),
                                         start=False, stop=False)
                        nc.tensor.matmul(zp, (wdhh[1][:, ms]), (hd[1]),
                                         start=False, stop=True)
                        zts.append(zp)
                    hd, cd = lstm_cell(zts, cd, bdt, False, stH, "Hd", stC, "Cd")
                    # output head
                    op_ = pq.tile([128, cols], F32, tag="pq")
                    nc.tensor.matmul(op_, (wo1[0]), (hd[0]),
                                     start=True, stop=False)
                    nc.tensor.matmul(op_, (wo1[1]), (hd[1]),
                                     start=False, stop=True)
                    ro = ap2.tile([128, cols], F32R, tag="ro")
                    nc.scalar.activation(ro, op_, AF.Relu, bias=bo1t[:, 0:1])
                    dlt = ps.tile([F, cols], F32, tag="ps")
                    nc.tensor.matmul(dlt, (wo2), (ro), start=True, stop=True)
                    db_ = ap4.tile([F, cols], F32, tag="at")
                    nc.scalar.activation(db_, dlt, AF.Identity, bias=bo2t)
                    pred = prd.tile([F, cols], F32R, tag="pred")
                    nc.vector.tensor_add(pred, lc, db_)
                    nc.sync.dma_start(out=dout[F * s:F * (s + 1), cs], in_=pred)
                    lc = pred
    nc.compile()
    return nc


_BUILD_CACHE = {}


def get_nc(n_bt=B_CORE // COLS, cols=COLS):
    key = (n_bt, cols)
    if key not in _BUILD_CACHE:
        _BUILD_CACHE[key] = build_nc(n_bt, cols)
    return _BUILD_CACHE[key]


def make_in_maps(inputs, n_cores=N_CORES, b_core=B_CORE):
    w = prep_weights(inputs)
    cbc = np.asarray(inputs["cbc_input"], np.float32)
    bas = np.asarray(inputs["baseline"], np.float32)
    in_maps = []
    for c in range(n_cores):
        sl = slice(c * b_core, (c + 1) * b_core)
        xc = np.ascontiguousarray(
            cbc[sl].transpose(1, 2, 0).reshape(T_IN * F, b_core))
        bc = np.ascontiguousarray(bas[sl].T)
        m = {"x": xc, "bas": bc}
        m.update(w)
        in_maps.append(m)
    return in_maps


def kernel(**inputs):
    global LAST_EXEC_NS, LAST_RESULTS
    import os
    nc = get_nc()
    in_maps = make_in_maps(inputs)
    trace = bool(int(os.environ.get("KERNEL_TRACE", "0")))
    res = run_bass_kernel_spmd(
        nc, in_maps, core_ids=list(range(N_CORES)), trace=trace)
    LAST_EXEC_NS = res.exec_time_ns
    LAST_RESULTS = res
    outs = []
    for c in range(N_CORES):
        o = res.results[c]["out"]  # [30, B_CORE]
        outs.append(o.reshape(T_OUT, F, B_CORE).transpose(2, 0, 1))
    return np.concatenate(outs, axis=0).astype(np.float32)
